# revision 22
# baseline (speedup 1.0000x reference)
"""Trainium2 Bass kernel for nn_LogicLayer (differentiable logic-gate layer).

Reference computation:
    a = x[:, idx_a]; b = x[:, idx_b]                  # [B, OUT] gathers
    w = softmax(weights, -1)                          # [OUT, 16]
    out = sum_k w[:, k] * gate_k(a, b)

Every gate value is of the form c0 + c1*a + c2*b + c3*a*b, so
    out[i, j] = W0[j] + W1[j]*a + W2[j]*b + W3[j]*a*b
with W = softmax(weights) @ C, C the [16, 4] gate-coefficient table.

Kernel strategy (out_dim-parallel across 8 cores, 1024 outputs/core,
full 2048-row batch per core):
  - host passes xT = x.T as fp16 [8192, 2048]; each gathered row is then
    4 KiB, so a core needs only 2048 gather descriptors total (SWDGE
    descriptor generation at ~8.5 ns/desc was the old bottleneck)
  - softmax+C projection on device -> W0..W3 [128, 8] tiles in SBUF
  - dma_gather rows of xT for idx_a / idx_b; out_dim lands on partitions
    (j = slot*128 + p), batch on the free axis
  - u = W3*a + W2 (ACT), v = W1*a + W0 (DVE ts), t = u*b (DVE tt),
    o = t + v (DVE tt), everything fp16
  - o stores straight to DRAM as outT [1024, 2048] fp16; host transposes
    back to [2048, OUT] f32 and concatenates the 8 core slices
"""

import numpy as np

# ---------------------------------------------------------------- constants
B_TOT, IN_DIM, OUT_DIM = 2048, 8192, 8192
NCORES = 8
OUT_SH = OUT_DIM // NCORES      # 1024 outputs per core
NSLOT = OUT_SH // 128           # 8 partition-slots per core
CHUNKS = (2, 2, 2, 2)           # slots per dma_gather call (sum = NSLOT)
NDVE_U = 2                      # trailing slots whose u-pass runs on DVE

# value = c0 + c1*a + c2*b + c3*ab  for each of the 16 gates
GATE_C = np.array(
    [
        # c0  c1  c2  c3
        [0, 0, 0, 0],    # 0  False
        [0, 0, 0, 1],    # 1  a AND b
        [0, 1, 0, -1],   # 2  a AND NOT b
        [0, 1, 0, 0],    # 3  a
        [0, 0, 1, -1],   # 4  NOT a AND b
        [0, 0, 1, 0],    # 5  b
        [0, 1, 1, -2],   # 6  a XOR b
        [0, 1, 1, -1],   # 7  a OR b
        [1, -1, -1, 1],  # 8  NOT (a OR b)
        [1, -1, -1, 2],  # 9  NOT (a XOR b)
        [1, 0, -1, 0],   # 10 NOT b
        [1, 0, -1, 1],   # 11 a OR NOT b
        [1, -1, 0, 0],   # 12 NOT a
        [1, -1, 0, 1],   # 13 NOT a OR b
        [1, 0, 0, -1],   # 14 NOT (a AND b)
        [1, 0, 0, 0],    # 15 True
    ],
    dtype=np.float32,
)  # [16, 4]


# ---------------------------------------------------------------- device IR
def build_nc(B=B_TOT, IN=IN_DIM, OSH=OUT_SH):
    """Build the per-core Bass module (SPMD; all cores run the same IR)."""
    import sys

    if "/opt/trn_rl_repo" not in sys.path:
        sys.path.insert(0, "/opt/trn_rl_repo")

    import concourse.tile as tile
    from concourse import bacc, mybir
    from contextlib import ExitStack

    f32 = mybir.dt.float32
    f16 = mybir.dt.float16
    i16 = mybir.dt.int16
    u8 = mybir.dt.uint8

    nc = bacc.Bacc("TRN2", target_bir_lowering=False)
    xT = nc.declare_dram_parameter("xT", [IN, B], f16, isOutput=False)
    wgt = nc.declare_dram_parameter("wgt_shuf", [128, NSLOT * 16], f32, isOutput=False)
    cg = nc.declare_dram_parameter("cgate", [128, 64], f32, isOutput=False)
    idxa = nc.declare_dram_parameter("idxa16", [128, OSH // 16], i16, isOutput=False)
    idxb = nc.declare_dram_parameter("idxb16", [128, OSH // 16], i16, isOutput=False)
    outT = nc.declare_dram_parameter("outT", [OSH // 2, 2 * B], u8, isOutput=True)

    Ident = mybir.ActivationFunctionType.Identity
    Exp = mybir.ActivationFunctionType.Exp
    MULT = mybir.AluOpType.mult
    ADD = mybir.AluOpType.add

    with tile.TileContext(nc) as tc, ExitStack() as ctx:
        from concourse import library_config
        nc.gpsimd.load_library(library_config.mlp)
        cpool = ctx.enter_context(tc.tile_pool(name="consts", bufs=1))
        wpool = ctx.enter_context(tc.tile_pool(name="wtmp", bufs=2))
        gpool = ctx.enter_context(tc.tile_pool(name="gath", bufs=1, side="right"))
        upool = ctx.enter_context(tc.tile_pool(name="u", bufs=6))
        vpool = ctx.enter_context(tc.tile_pool(name="v", bufs=6))
        tpool = ctx.enter_context(tc.tile_pool(name="t", bufs=3))
        opool = ctx.enter_context(tc.tile_pool(name="o", bufs=4))

        # idx loads first: the gather chain is the critical path
        idxa_sb = cpool.tile([128, OSH // 16], i16, name="idxa_sb")
        nc.sync.dma_start(idxa_sb[:], idxa[:])
        idxb_sb = cpool.tile([128, OSH // 16], i16, name="idxb_sb")
        nc.sync.dma_start(idxb_sb[:], idxb[:])
        cgt = cpool.tile([128, 64], f32, name="cgt")
        nc.sync.dma_start(cgt[:], cg[:])
        wtile = wpool.tile([128, NSLOT * 16], f32, name="wtile")
        nc.sync.dma_start(wtile[:], wgt[:])

        # ---- gathers launch first (longest dependency chain) -------------
        # j = (s0 + c)*128 + p lands at ga[p, c, :]
        gt = {}
        s0 = 0
        for ck, CH in enumerate(CHUNKS):
            NJ = CH * 128
            ga = gpool.tile([128, CH, B], f16, name=f"ga{ck}", tag=f"ga{ck}")
            nc.gpsimd.dma_gather(
                ga[:], xT[:], idxa_sb[:, s0 * 8:(s0 + CH) * 8],
                NJ, NJ, B,
            )
            gb = gpool.tile([128, CH, B], f16, name=f"gb{ck}", tag=f"gb{ck}")
            nc.gpsimd.dma_gather(
                gb[:], xT[:], idxb_sb[:, s0 * 8:(s0 + CH) * 8],
                NJ, NJ, B,
            )
            gt[ck] = (ga, gb)
            s0 += CH

            # ---- W = softmax(weights) @ C, layout wk[k][q, r], j = r*128+q
            # (issued after the first chunk's gathers so SWDGE gen leads)
            if ck == 0:
                # W-phase reduces on GpSimd: it is idle until the first
                # dma_gather unblocks (fixed ~16.6us into the kernel)
                wexp = wpool.tile([128, NSLOT * 16], f32, name="wexp")
                nc.scalar.activation(wexp[:], wtile[:], Exp)
                wsum = wpool.tile([128, NSLOT], f32, name="wsum")
                nc.vector.tensor_reduce(
                    out=wsum[:],
                    in_=wexp[:].rearrange("p (r k) -> p r k", k=16),
                    op=ADD,
                    axis=mybir.AxisListType.X,
                )
                wrcp = wpool.tile([128, NSLOT], f32, name="wrcp")
                nc.vector.reciprocal(wrcp[:], wsum[:])
                wk = [cpool.tile([128, NSLOT], f32, name=f"wk{k}") for k in range(4)]
                for k in range(4):
                    wtmp = wpool.tile([128, NSLOT * 16], f32, name="wtmp", tag="wtmp")
                    ck_bcast = (
                        cgt[:, k * 16:(k + 1) * 16]
                        .rearrange("p (r k) -> p r k", r=1)
                        .to_broadcast([128, NSLOT, 16])
                    )
                    nc.vector.tensor_tensor(
                        out=wtmp[:].rearrange("p (r k) -> p r k", k=16),
                        in0=wexp[:].rearrange("p (r k) -> p r k", k=16),
                        in1=ck_bcast,
                        op=MULT,
                    )
                    wred = wpool.tile([128, NSLOT], f32, name="wred", tag="wred")
                    nc.vector.tensor_reduce(
                        out=wred[:],
                        in_=wtmp[:].rearrange("p (r k) -> p r k", k=16),
                        op=ADD,
                        axis=mybir.AxisListType.X,
                    )
                    nc.vector.tensor_tensor(out=wk[k][:], in0=wred[:], in1=wrcp[:],
                                            op=MULT)

        # ---- gates: out = (W1*a + W0) + (W3*a + W2)*b --------------------
        s0 = 0
        for ck, CH in enumerate(CHUNKS):
            ga, gb = gt[ck]
            for c in range(CH):
                r = s0 + c
                u = upool.tile([128, B], f16, tag="u")
                if r >= NSLOT - NDVE_U:
                    # tail slots: keep the whole chain on DVE so the final
                    # slots don't wait on ACT's slow serial u stream
                    nc.vector.tensor_scalar(
                        u[:], ga[:, c, :],
                        wk[3][:, r:r + 1], wk[2][:, r:r + 1],
                        op0=MULT, op1=ADD,
                    )
                else:
                    nc.scalar.activation(
                        u[:], ga[:, c, :], Ident,
                        scale=wk[3][:, r:r + 1], bias=wk[2][:, r:r + 1],
                    )
                v = vpool.tile([128, B], f16, tag="v")
                nc.vector.tensor_scalar(
                    v[:], ga[:, c, :],
                    wk[1][:, r:r + 1], wk[0][:, r:r + 1],
                    op0=MULT, op1=ADD,
                )
                t = tpool.tile([128, B], f16, tag="t")
                nc.vector.tensor_tensor(t[:], u[:], gb[:, c, :], op=MULT)
                if c % 2 == 0:
                    po = opool.tile([128, 2, B], u8, tag="o")
                # o8 = 252*out + 1 (+1 guards fp16 slop against u8 wrap);
                # v carries the 252/1.0 scales from the host cgate
                nc.vector.scalar_tensor_tensor(
                    po[:, c % 2, :], t[:], 252.0, v[:], op0=MULT, op1=ADD,
                )
                if c % 2 == 1:
                    q = r // 2
                    nc.sync.dma_start(
                        outT[q * 128:(q + 1) * 128, :],
                        po[:].rearrange("p i b -> p (i b)"),
                    )
            s0 += CH
    nc.compile()
    return nc


# ---------------------------------------------------------------- host side
def _wrap_idx(idx, n):
    """Pack an index vector into dma_gather's wrapped int16 layout.

    idx16[p, s] = idx[s*16 + p%16], replicated over the 8 groups of 16
    partitions; a gather of NJ indices starting at slot s0 then reads
    columns [s0*8, s0*8 + NJ/16).
    """
    a = np.asarray(idx).astype(np.int16).reshape(n // 16, 16)  # [s, p]
    a = np.ascontiguousarray(a.T)                              # [16, s]
    return np.ascontiguousarray(np.tile(a, (8, 1)))            # [128, s]


def _prep_inputs(x, weights, idx_a, idx_b):
    x = np.asarray(x, dtype=np.float32)
    weights = np.asarray(weights, dtype=np.float32)
    idx_a = np.asarray(idx_a)
    idx_b = np.asarray(idx_b)
    xT16 = np.ascontiguousarray(x.astype(np.float16).T)  # [IN, B] fp16
    # output leaves the device as o8 = 252*out + 1 (uint8, slot-paired
    # stores keep 4KiB DMA descriptors); fold the affine map into the
    # v-pass coefficients: v = 252*(W1*a + W0) + 1
    gc = GATE_C.copy()
    gc[:, 0] = 252.0 * gc[:, 0] + 1.0
    gc[:, 1] *= 252.0
    cgate = np.ascontiguousarray(np.tile(gc.T.reshape(1, 64), (128, 1)))
    in_maps = []
    for c in range(NCORES):
        j0 = c * OUT_SH
        wsh = weights[j0:j0 + OUT_SH]  # [1024, 16]
        # wgt_shuf[q, r*16+k] = weights[j0 + r*128 + q, k]
        wgt_shuf = np.ascontiguousarray(
            wsh.reshape(NSLOT, 128, 16).transpose(1, 0, 2).reshape(128, -1)
        )
        in_maps.append(
            {
                "xT": xT16,
                "wgt_shuf": wgt_shuf,
                "cgate": cgate,
                "idxa16": _wrap_idx(idx_a[j0:j0 + OUT_SH], OUT_SH),
                "idxb16": _wrap_idx(idx_b[j0:j0 + OUT_SH], OUT_SH),
            }
        )
    return in_maps


def _assemble(results):
    """Packed [OUT_SH//2, 2*B] uint8 per core -> full [B, OUT] f32.

    Row q of a core's outT holds j_even = (q//128)*256 + q%128 at columns
    [0,B) and j_odd = j_even + 128 at [B,2B); o8 = 252*out + 1 with a
    floor-vs-round-agnostic decode offset of 0.75.
    """
    cores = []
    for r in results:
        a = np.asarray(r["outT"]).reshape(NSLOT // 2, 128, 2, B_TOT)
        cores.append(a.transpose(0, 2, 1, 3).reshape(OUT_SH, B_TOT))
    stacked = np.stack(cores)  # [8, 1024, 2048]
    out = (stacked.astype(np.float32) - 0.75) * np.float32(1.0 / 252.0)
    return np.ascontiguousarray(out.transpose(2, 0, 1).reshape(B_TOT, OUT_DIM))


_NC_CACHE = {}


def _get_nc():
    if "nc" not in _NC_CACHE:
        _NC_CACHE["nc"] = build_nc()
    return _NC_CACHE["nc"]


def kernel(x, weights, idx_a, idx_b):
    import sys

    if "/opt/trn_rl_repo" not in sys.path:
        sys.path.insert(0, "/opt/trn_rl_repo")
    from concourse.bass_utils import run_bass_kernel_spmd

    nc = _get_nc()
    in_maps = _prep_inputs(x, weights, idx_a, idx_b)
    res = run_bass_kernel_spmd(nc, in_maps, list(range(NCORES)))
    return _assemble(res.results)


if __name__ == "__main__":
    nc = build_nc()
    print("built OK")


# revision 23
# speedup vs baseline: 1.2067x; 1.2067x over previous
"""Trainium2 Bass kernel for nn_LogicLayer (differentiable logic-gate layer).

Reference computation:
    a = x[:, idx_a]; b = x[:, idx_b]                  # [B, OUT] gathers
    w = softmax(weights, -1)                          # [OUT, 16]
    out = sum_k w[:, k] * gate_k(a, b)

Every gate value is of the form c0 + c1*a + c2*b + c3*a*b, so
    out[i, j] = W0[j] + W1[j]*a + W2[j]*b + W3[j]*a*b
with W = softmax(weights) @ C, C the [16, 4] gate-coefficient table.

Kernel strategy (out_dim-parallel across 8 cores, 1024 outputs/core,
full 2048-row batch per core):
  - host passes xT = x.T as fp16 [8192, 2048]; each gathered row is then
    4 KiB, so a core needs only 2048 gather descriptors total (SWDGE
    descriptor generation at ~8.5 ns/desc was the old bottleneck)
  - softmax+C projection on device -> W0..W3 [128, 8] tiles in SBUF
  - dma_gather rows of xT for idx_a / idx_b; out_dim lands on partitions
    (j = slot*128 + p), batch on the free axis
  - u = W3*a + W2 (ACT), v = W1*a + W0 (DVE ts), t = u*b (DVE tt),
    o = t + v (DVE tt), everything fp16
  - o stores straight to DRAM as outT [1024, 2048] fp16; host transposes
    back to [2048, OUT] f32 and concatenates the 8 core slices
"""

import numpy as np

# ---------------------------------------------------------------- constants
B_TOT, IN_DIM, OUT_DIM = 2048, 8192, 8192
NCORES = 8
OUT_SH = OUT_DIM // NCORES      # 1024 outputs per core
NSLOT = OUT_SH // 128           # 8 partition-slots per core
CHUNKS = (2, 2, 2, 2)           # slots per dma_gather call (sum = NSLOT)
NDVE_U = 2                      # trailing slots whose u-pass runs on DVE

# value = c0 + c1*a + c2*b + c3*ab  for each of the 16 gates
GATE_C = np.array(
    [
        # c0  c1  c2  c3
        [0, 0, 0, 0],    # 0  False
        [0, 0, 0, 1],    # 1  a AND b
        [0, 1, 0, -1],   # 2  a AND NOT b
        [0, 1, 0, 0],    # 3  a
        [0, 0, 1, -1],   # 4  NOT a AND b
        [0, 0, 1, 0],    # 5  b
        [0, 1, 1, -2],   # 6  a XOR b
        [0, 1, 1, -1],   # 7  a OR b
        [1, -1, -1, 1],  # 8  NOT (a OR b)
        [1, -1, -1, 2],  # 9  NOT (a XOR b)
        [1, 0, -1, 0],   # 10 NOT b
        [1, 0, -1, 1],   # 11 a OR NOT b
        [1, -1, 0, 0],   # 12 NOT a
        [1, -1, 0, 1],   # 13 NOT a OR b
        [1, 0, 0, -1],   # 14 NOT (a AND b)
        [1, 0, 0, 0],    # 15 True
    ],
    dtype=np.float32,
)  # [16, 4]


# ---------------------------------------------------------------- device IR
def build_nc(B=B_TOT, IN=IN_DIM, OSH=OUT_SH):
    """Build the per-core Bass module (SPMD; all cores run the same IR)."""
    import sys

    if "/opt/trn_rl_repo" not in sys.path:
        sys.path.insert(0, "/opt/trn_rl_repo")

    import concourse.tile as tile
    from concourse import bacc, mybir
    from contextlib import ExitStack

    f32 = mybir.dt.float32
    f16 = mybir.dt.float16
    i16 = mybir.dt.int16
    u8 = mybir.dt.uint8

    nc = bacc.Bacc("TRN2", target_bir_lowering=False)
    xT = nc.declare_dram_parameter("xT", [IN, B], f16, isOutput=False)
    wgt = nc.declare_dram_parameter("wgt_shuf", [128, NSLOT * 16], f32, isOutput=False)
    cg = nc.declare_dram_parameter("cgate", [128, 64], f32, isOutput=False)
    idxa = nc.declare_dram_parameter("idxa16", [128, OSH // 16], i16, isOutput=False)
    idxb = nc.declare_dram_parameter("idxb16", [128, OSH // 16], i16, isOutput=False)
    outT = nc.declare_dram_parameter("outT", [OSH, B], f16, isOutput=True)

    Ident = mybir.ActivationFunctionType.Identity
    Exp = mybir.ActivationFunctionType.Exp
    MULT = mybir.AluOpType.mult
    ADD = mybir.AluOpType.add

    with tile.TileContext(nc) as tc, ExitStack() as ctx:
        cpool = ctx.enter_context(tc.tile_pool(name="consts", bufs=1))
        wpool = ctx.enter_context(tc.tile_pool(name="wtmp", bufs=2))
        gpool = ctx.enter_context(tc.tile_pool(name="gath", bufs=1, side="right"))
        upool = ctx.enter_context(tc.tile_pool(name="u", bufs=6))
        vpool = ctx.enter_context(tc.tile_pool(name="v", bufs=6))
        tpool = ctx.enter_context(tc.tile_pool(name="t", bufs=3))
        opool = ctx.enter_context(tc.tile_pool(name="o", bufs=4))

        # idx loads first: the gather chain is the critical path
        idxa_sb = cpool.tile([128, OSH // 16], i16, name="idxa_sb")
        nc.sync.dma_start(idxa_sb[:], idxa[:])
        idxb_sb = cpool.tile([128, OSH // 16], i16, name="idxb_sb")
        nc.sync.dma_start(idxb_sb[:], idxb[:])
        cgt = cpool.tile([128, 64], f32, name="cgt")
        nc.sync.dma_start(cgt[:], cg[:])
        wtile = wpool.tile([128, NSLOT * 16], f32, name="wtile")
        nc.sync.dma_start(wtile[:], wgt[:])

        # ---- gathers launch first (longest dependency chain) -------------
        # j = (s0 + c)*128 + p lands at ga[p, c, :]
        gt = {}
        s0 = 0
        for ck, CH in enumerate(CHUNKS):
            NJ = CH * 128
            ga = gpool.tile([128, CH, B], f16, name=f"ga{ck}", tag=f"ga{ck}")
            nc.gpsimd.dma_gather(
                ga[:], xT[:], idxa_sb[:, s0 * 8:(s0 + CH) * 8],
                NJ, NJ, B,
            )
            gb = gpool.tile([128, CH, B], f16, name=f"gb{ck}", tag=f"gb{ck}")
            nc.gpsimd.dma_gather(
                gb[:], xT[:], idxb_sb[:, s0 * 8:(s0 + CH) * 8],
                NJ, NJ, B,
            )
            gt[ck] = (ga, gb)
            s0 += CH

            # ---- W = softmax(weights) @ C, layout wk[k][q, r], j = r*128+q
            # (issued after the first chunk's gathers so SWDGE gen leads)
            if ck == 0:
                # W-phase reduces on GpSimd: it is idle until the first
                # dma_gather unblocks (fixed ~16.6us into the kernel)
                wexp = wpool.tile([128, NSLOT * 16], f32, name="wexp")
                nc.scalar.activation(wexp[:], wtile[:], Exp)
                wsum = wpool.tile([128, NSLOT], f32, name="wsum")
                nc.vector.tensor_reduce(
                    out=wsum[:],
                    in_=wexp[:].rearrange("p (r k) -> p r k", k=16),
                    op=ADD,
                    axis=mybir.AxisListType.X,
                )
                wrcp = wpool.tile([128, NSLOT], f32, name="wrcp")
                nc.vector.reciprocal(wrcp[:], wsum[:])
                wk = [cpool.tile([128, NSLOT], f32, name=f"wk{k}") for k in range(4)]
                for k in range(4):
                    wtmp = wpool.tile([128, NSLOT * 16], f32, name="wtmp", tag="wtmp")
                    ck_bcast = (
                        cgt[:, k * 16:(k + 1) * 16]
                        .rearrange("p (r k) -> p r k", r=1)
                        .to_broadcast([128, NSLOT, 16])
                    )
                    nc.vector.tensor_tensor(
                        out=wtmp[:].rearrange("p (r k) -> p r k", k=16),
                        in0=wexp[:].rearrange("p (r k) -> p r k", k=16),
                        in1=ck_bcast,
                        op=MULT,
                    )
                    wred = wpool.tile([128, NSLOT], f32, name="wred", tag="wred")
                    nc.vector.tensor_reduce(
                        out=wred[:],
                        in_=wtmp[:].rearrange("p (r k) -> p r k", k=16),
                        op=ADD,
                        axis=mybir.AxisListType.X,
                    )
                    nc.vector.tensor_tensor(out=wk[k][:], in0=wred[:], in1=wrcp[:],
                                            op=MULT)

        # ---- gates: out = (W1*a + W0) + (W3*a + W2)*b --------------------
        s0 = 0
        for ck, CH in enumerate(CHUNKS):
            ga, gb = gt[ck]
            for c in range(CH):
                r = s0 + c
                u = upool.tile([128, B], f16, tag="u")
                if r >= NSLOT - NDVE_U:
                    # tail slots: keep the whole chain on DVE so the final
                    # slots don't wait on ACT's slow serial u stream
                    nc.vector.tensor_scalar(
                        u[:], ga[:, c, :],
                        wk[3][:, r:r + 1], wk[2][:, r:r + 1],
                        op0=MULT, op1=ADD,
                    )
                else:
                    nc.scalar.activation(
                        u[:], ga[:, c, :], Ident,
                        scale=wk[3][:, r:r + 1], bias=wk[2][:, r:r + 1],
                    )
                v = vpool.tile([128, B], f16, tag="v")
                nc.vector.tensor_scalar(
                    v[:], ga[:, c, :],
                    wk[1][:, r:r + 1], wk[0][:, r:r + 1],
                    op0=MULT, op1=ADD,
                )
                t = tpool.tile([128, B], f16, tag="t")
                nc.vector.tensor_tensor(t[:], u[:], gb[:, c, :], op=MULT)
                o = opool.tile([128, B], f16, tag="o")
                nc.vector.tensor_tensor(o[:], t[:], v[:], op=ADD)
                nc.sync.dma_start(outT[r * 128:(r + 1) * 128, :], o[:])
            s0 += CH
    nc.compile()
    return nc


# ---------------------------------------------------------------- host side
def _wrap_idx(idx, n):
    """Pack an index vector into dma_gather's wrapped int16 layout.

    idx16[p, s] = idx[s*16 + p%16], replicated over the 8 groups of 16
    partitions; a gather of NJ indices starting at slot s0 then reads
    columns [s0*8, s0*8 + NJ/16).
    """
    a = np.asarray(idx).astype(np.int16).reshape(n // 16, 16)  # [s, p]
    a = np.ascontiguousarray(a.T)                              # [16, s]
    return np.ascontiguousarray(np.tile(a, (8, 1)))            # [128, s]


def _prep_inputs(x, weights, idx_a, idx_b):
    x = np.asarray(x, dtype=np.float32)
    weights = np.asarray(weights, dtype=np.float32)
    idx_a = np.asarray(idx_a)
    idx_b = np.asarray(idx_b)
    xT16 = np.ascontiguousarray(x.astype(np.float16).T)  # [IN, B] fp16
    cgate = np.ascontiguousarray(np.tile(GATE_C.T.reshape(1, 64), (128, 1)))
    in_maps = []
    for c in range(NCORES):
        j0 = c * OUT_SH
        wsh = weights[j0:j0 + OUT_SH]  # [1024, 16]
        # wgt_shuf[q, r*16+k] = weights[j0 + r*128 + q, k]
        wgt_shuf = np.ascontiguousarray(
            wsh.reshape(NSLOT, 128, 16).transpose(1, 0, 2).reshape(128, -1)
        )
        in_maps.append(
            {
                "xT": xT16,
                "wgt_shuf": wgt_shuf,
                "cgate": cgate,
                "idxa16": _wrap_idx(idx_a[j0:j0 + OUT_SH], OUT_SH),
                "idxb16": _wrap_idx(idx_b[j0:j0 + OUT_SH], OUT_SH),
            }
        )
    return in_maps


def _assemble(results):
    """[OUT_SH, B] fp16 per core -> full [B, OUT] f32."""
    stacked = np.stack([np.asarray(r["outT"]) for r in results])  # [8, 1024, 2048]
    return np.ascontiguousarray(
        stacked.astype(np.float32).transpose(2, 0, 1).reshape(B_TOT, OUT_DIM)
    )


_NC_CACHE = {}


def _get_nc():
    if "nc" not in _NC_CACHE:
        _NC_CACHE["nc"] = build_nc()
    return _NC_CACHE["nc"]


def kernel(x, weights, idx_a, idx_b):
    import sys

    if "/opt/trn_rl_repo" not in sys.path:
        sys.path.insert(0, "/opt/trn_rl_repo")
    from concourse.bass_utils import run_bass_kernel_spmd

    nc = _get_nc()
    in_maps = _prep_inputs(x, weights, idx_a, idx_b)
    res = run_bass_kernel_spmd(nc, in_maps, list(range(NCORES)))
    return _assemble(res.results)


if __name__ == "__main__":
    nc = build_nc()
    print("built OK")


# revision 27
# speedup vs baseline: 1.2263x; 1.0163x over previous
"""Trainium2 Bass kernel for nn_LogicLayer (differentiable logic-gate layer).

Reference computation:
    a = x[:, idx_a]; b = x[:, idx_b]                  # [B, OUT] gathers
    w = softmax(weights, -1)                          # [OUT, 16]
    out = sum_k w[:, k] * gate_k(a, b)

Every gate value is of the form c0 + c1*a + c2*b + c3*a*b, so
    out[i, j] = W0[j] + W1[j]*a + W2[j]*b + W3[j]*a*b
with W = softmax(weights) @ C, C the [16, 4] gate-coefficient table.

Kernel strategy (out_dim-parallel across 8 cores, 1024 outputs/core,
full 2048-row batch per core):
  - host passes xT = x.T as fp16 [8192, 2048]; each gathered row is then
    4 KiB, so a core needs only 2048 gather descriptors total (SWDGE
    descriptor generation at ~8.5 ns/desc was the old bottleneck)
  - softmax+C projection on device -> W0..W3 [128, 8] tiles in SBUF
  - dma_gather rows of xT for idx_a / idx_b; out_dim lands on partitions
    (j = slot*128 + p), batch on the free axis
  - u = W3*a + W2 (ACT), v = W1*a + W0 (DVE ts), t = u*b (DVE tt),
    o = t + v (DVE tt), everything fp16
  - o stores straight to DRAM as outT [1024, 2048] fp16; host transposes
    back to [2048, OUT] f32 and concatenates the 8 core slices
"""

import numpy as np

# ---------------------------------------------------------------- constants
B_TOT, IN_DIM, OUT_DIM = 2048, 8192, 8192
NCORES = 8
OUT_SH = OUT_DIM // NCORES      # 1024 outputs per core
NSLOT = OUT_SH // 128           # 8 partition-slots per core
CHUNKS = (2, 2, 2, 2)           # slots per dma_gather call (sum = NSLOT)
GATHER_ORDER = (0, 3, 1, 2)     # chunk gather issue order
COMPUTE_ORDER = (0, 1, 6, 7, 2, 3, 4, 5)  # slot compute order
NDVE_U = 2                      # last computed slots: whole chain on DVE

# value = c0 + c1*a + c2*b + c3*ab  for each of the 16 gates
GATE_C = np.array(
    [
        # c0  c1  c2  c3
        [0, 0, 0, 0],    # 0  False
        [0, 0, 0, 1],    # 1  a AND b
        [0, 1, 0, -1],   # 2  a AND NOT b
        [0, 1, 0, 0],    # 3  a
        [0, 0, 1, -1],   # 4  NOT a AND b
        [0, 0, 1, 0],    # 5  b
        [0, 1, 1, -2],   # 6  a XOR b
        [0, 1, 1, -1],   # 7  a OR b
        [1, -1, -1, 1],  # 8  NOT (a OR b)
        [1, -1, -1, 2],  # 9  NOT (a XOR b)
        [1, 0, -1, 0],   # 10 NOT b
        [1, 0, -1, 1],   # 11 a OR NOT b
        [1, -1, 0, 0],   # 12 NOT a
        [1, -1, 0, 1],   # 13 NOT a OR b
        [1, 0, 0, -1],   # 14 NOT (a AND b)
        [1, 0, 0, 0],    # 15 True
    ],
    dtype=np.float32,
)  # [16, 4]


# ---------------------------------------------------------------- device IR
def build_nc(B=B_TOT, IN=IN_DIM, OSH=OUT_SH):
    """Build the per-core Bass module (SPMD; all cores run the same IR)."""
    import sys

    if "/opt/trn_rl_repo" not in sys.path:
        sys.path.insert(0, "/opt/trn_rl_repo")

    import concourse.tile as tile
    from concourse import bacc, mybir
    from contextlib import ExitStack

    f32 = mybir.dt.float32
    f16 = mybir.dt.float16
    i16 = mybir.dt.int16
    u8 = mybir.dt.uint8

    nc = bacc.Bacc("TRN2", target_bir_lowering=False)
    xT = nc.declare_dram_parameter("xT", [IN, B], f16, isOutput=False)
    wgt = nc.declare_dram_parameter("wgt_shuf", [128, NSLOT * 16], f32, isOutput=False)
    cg = nc.declare_dram_parameter("cgate", [128, 64], f32, isOutput=False)
    idxa = nc.declare_dram_parameter("idxa16", [128, OSH // 16], i16, isOutput=False)
    idxb = nc.declare_dram_parameter("idxb16", [128, OSH // 16], i16, isOutput=False)
    outT = nc.declare_dram_parameter("outT", [OSH, B], f16, isOutput=True)

    Ident = mybir.ActivationFunctionType.Identity
    Exp = mybir.ActivationFunctionType.Exp
    MULT = mybir.AluOpType.mult
    ADD = mybir.AluOpType.add

    with tile.TileContext(nc) as tc, ExitStack() as ctx:
        cpool = ctx.enter_context(tc.tile_pool(name="consts", bufs=1))
        wpool = ctx.enter_context(tc.tile_pool(name="wtmp", bufs=2))
        gpool = ctx.enter_context(tc.tile_pool(name="gath", bufs=1, side="right"))
        upool = ctx.enter_context(tc.tile_pool(name="u", bufs=6))
        vpool = ctx.enter_context(tc.tile_pool(name="v", bufs=6))
        tpool = ctx.enter_context(tc.tile_pool(name="t", bufs=3))
        opool = ctx.enter_context(tc.tile_pool(name="o", bufs=4))

        # idx loads first: the gather chain is the critical path
        idxa_sb = cpool.tile([128, OSH // 16], i16, name="idxa_sb")
        nc.sync.dma_start(idxa_sb[:], idxa[:])
        idxb_sb = cpool.tile([128, OSH // 16], i16, name="idxb_sb")
        nc.sync.dma_start(idxb_sb[:], idxb[:])
        cgt = cpool.tile([128, 64], f32, name="cgt")
        nc.sync.dma_start(cgt[:], cg[:])
        wtile = wpool.tile([128, NSLOT * 16], f32, name="wtile")
        nc.sync.dma_start(wtile[:], wgt[:])

        # ---- gathers launch first (longest dependency chain) -------------
        # j = (s0 + c)*128 + p lands at ga[p, c, :]. The LAST-computed
        # chunks are gathered early (GATHER_ORDER) so the compute tail is
        # never data-starved and the kernel ends on the store stream.
        gt = {}
        for gi, ck in enumerate(GATHER_ORDER):
            CH = CHUNKS[ck]
            s0 = sum(CHUNKS[:ck])
            NJ = CH * 128
            ga = gpool.tile([128, CH, B], f16, name=f"ga{ck}", tag=f"ga{ck}")
            nc.gpsimd.dma_gather(
                ga[:], xT[:], idxa_sb[:, s0 * 8:(s0 + CH) * 8],
                NJ, NJ, B,
            )
            gb = gpool.tile([128, CH, B], f16, name=f"gb{ck}", tag=f"gb{ck}")
            nc.gpsimd.dma_gather(
                gb[:], xT[:], idxb_sb[:, s0 * 8:(s0 + CH) * 8],
                NJ, NJ, B,
            )
            gt[ck] = (ga, gb)

            # ---- W = softmax(weights) @ C, layout wk[k][q, r], j = r*128+q
            # (issued after the first chunk's gathers so SWDGE gen leads)
            if gi == 0:
                # W-phase reduces on GpSimd: it is idle until the first
                # dma_gather unblocks (fixed ~16.6us into the kernel)
                wexp = wpool.tile([128, NSLOT * 16], f32, name="wexp")
                nc.scalar.activation(wexp[:], wtile[:], Exp)
                wsum = wpool.tile([128, NSLOT], f32, name="wsum")
                nc.vector.tensor_reduce(
                    out=wsum[:],
                    in_=wexp[:].rearrange("p (r k) -> p r k", k=16),
                    op=ADD,
                    axis=mybir.AxisListType.X,
                )
                wrcp = wpool.tile([128, NSLOT], f32, name="wrcp")
                nc.vector.reciprocal(wrcp[:], wsum[:])
                wk = [cpool.tile([128, NSLOT], f32, name=f"wk{k}") for k in range(4)]
                for k in range(4):
                    wtmp = wpool.tile([128, NSLOT * 16], f32, name="wtmp", tag="wtmp")
                    ck_bcast = (
                        cgt[:, k * 16:(k + 1) * 16]
                        .rearrange("p (r k) -> p r k", r=1)
                        .to_broadcast([128, NSLOT, 16])
                    )
                    nc.vector.tensor_tensor(
                        out=wtmp[:].rearrange("p (r k) -> p r k", k=16),
                        in0=wexp[:].rearrange("p (r k) -> p r k", k=16),
                        in1=ck_bcast,
                        op=MULT,
                    )
                    wred = wpool.tile([128, NSLOT], f32, name="wred", tag="wred")
                    nc.vector.tensor_reduce(
                        out=wred[:],
                        in_=wtmp[:].rearrange("p (r k) -> p r k", k=16),
                        op=ADD,
                        axis=mybir.AxisListType.X,
                    )
                    nc.vector.tensor_tensor(out=wk[k][:], in0=wred[:], in1=wrcp[:],
                                            op=MULT)

        # ---- gates: out = (W1*a + W0) + (W3*a + W2)*b --------------------
        for ci, r in enumerate(COMPUTE_ORDER):
            ck, c = r // 2, r % 2
            ga, gb = gt[ck]
            if True:
                u = upool.tile([128, B], f16, tag="u")
                if ci >= len(COMPUTE_ORDER) - NDVE_U:
                    # tail slots: keep the whole chain on DVE so the final
                    # slots don't wait on ACT's slow serial u stream
                    nc.vector.tensor_scalar(
                        u[:], ga[:, c, :],
                        wk[3][:, r:r + 1], wk[2][:, r:r + 1],
                        op0=MULT, op1=ADD,
                    )
                else:
                    nc.scalar.activation(
                        u[:], ga[:, c, :], Ident,
                        scale=wk[3][:, r:r + 1], bias=wk[2][:, r:r + 1],
                    )
                v = vpool.tile([128, B], f16, tag="v")
                nc.vector.tensor_scalar(
                    v[:], ga[:, c, :],
                    wk[1][:, r:r + 1], wk[0][:, r:r + 1],
                    op0=MULT, op1=ADD,
                )
                t = tpool.tile([128, B], f16, tag="t")
                nc.vector.tensor_tensor(t[:], u[:], gb[:, c, :], op=MULT)
                o = opool.tile([128, B], f16, tag="o")
                nc.vector.tensor_tensor(o[:], t[:], v[:], op=ADD)
                nc.sync.dma_start(outT[r * 128:(r + 1) * 128, :], o[:])
    nc.compile()
    return nc


# ---------------------------------------------------------------- host side
def _wrap_idx(idx, n):
    """Pack an index vector into dma_gather's wrapped int16 layout.

    idx16[p, s] = idx[s*16 + p%16], replicated over the 8 groups of 16
    partitions; a gather of NJ indices starting at slot s0 then reads
    columns [s0*8, s0*8 + NJ/16).
    """
    a = np.asarray(idx).astype(np.int16).reshape(n // 16, 16)  # [s, p]
    a = np.ascontiguousarray(a.T)                              # [16, s]
    return np.ascontiguousarray(np.tile(a, (8, 1)))            # [128, s]


def _prep_inputs(x, weights, idx_a, idx_b):
    x = np.asarray(x, dtype=np.float32)
    weights = np.asarray(weights, dtype=np.float32)
    idx_a = np.asarray(idx_a)
    idx_b = np.asarray(idx_b)
    xT16 = np.ascontiguousarray(x.astype(np.float16).T)  # [IN, B] fp16
    cgate = np.ascontiguousarray(np.tile(GATE_C.T.reshape(1, 64), (128, 1)))
    in_maps = []
    for c in range(NCORES):
        j0 = c * OUT_SH
        wsh = weights[j0:j0 + OUT_SH]  # [1024, 16]
        # wgt_shuf[q, r*16+k] = weights[j0 + r*128 + q, k]
        wgt_shuf = np.ascontiguousarray(
            wsh.reshape(NSLOT, 128, 16).transpose(1, 0, 2).reshape(128, -1)
        )
        in_maps.append(
            {
                "xT": xT16,
                "wgt_shuf": wgt_shuf,
                "cgate": cgate,
                "idxa16": _wrap_idx(idx_a[j0:j0 + OUT_SH], OUT_SH),
                "idxb16": _wrap_idx(idx_b[j0:j0 + OUT_SH], OUT_SH),
            }
        )
    return in_maps


def _assemble(results):
    """[OUT_SH, B] fp16 per core -> full [B, OUT] f32."""
    stacked = np.stack([np.asarray(r["outT"]) for r in results])  # [8, 1024, 2048]
    return np.ascontiguousarray(
        stacked.astype(np.float32).transpose(2, 0, 1).reshape(B_TOT, OUT_DIM)
    )


_NC_CACHE = {}


def _get_nc():
    if "nc" not in _NC_CACHE:
        _NC_CACHE["nc"] = build_nc()
    return _NC_CACHE["nc"]


def kernel(x, weights, idx_a, idx_b):
    import sys

    if "/opt/trn_rl_repo" not in sys.path:
        sys.path.insert(0, "/opt/trn_rl_repo")
    from concourse.bass_utils import run_bass_kernel_spmd

    nc = _get_nc()
    in_maps = _prep_inputs(x, weights, idx_a, idx_b)
    res = run_bass_kernel_spmd(nc, in_maps, list(range(NCORES)))
    return _assemble(res.results)


if __name__ == "__main__":
    nc = build_nc()
    print("built OK")


# revision 28
# speedup vs baseline: 1.2362x; 1.0081x over previous
"""Trainium2 Bass kernel for nn_LogicLayer (differentiable logic-gate layer).

Reference computation:
    a = x[:, idx_a]; b = x[:, idx_b]                  # [B, OUT] gathers
    w = softmax(weights, -1)                          # [OUT, 16]
    out = sum_k w[:, k] * gate_k(a, b)

Every gate value is of the form c0 + c1*a + c2*b + c3*a*b, so
    out[i, j] = W0[j] + W1[j]*a + W2[j]*b + W3[j]*a*b
with W = softmax(weights) @ C, C the [16, 4] gate-coefficient table.

Kernel strategy (out_dim-parallel across 8 cores, 1024 outputs/core,
full 2048-row batch per core):
  - host passes xT = x.T as fp16 [8192, 2048]; each gathered row is then
    4 KiB, so a core needs only 2048 gather descriptors total (SWDGE
    descriptor generation at ~8.5 ns/desc was the old bottleneck)
  - softmax+C projection on device -> W0..W3 [128, 8] tiles in SBUF
  - dma_gather rows of xT for idx_a / idx_b; out_dim lands on partitions
    (j = slot*128 + p), batch on the free axis
  - u = W3*a + W2 (ACT), v = W1*a + W0 (DVE ts), t = u*b (DVE tt),
    o = t + v (DVE tt), everything fp16
  - o stores straight to DRAM as outT [1024, 2048] fp16; host transposes
    back to [2048, OUT] f32 and concatenates the 8 core slices
"""

import numpy as np

# ---------------------------------------------------------------- constants
B_TOT, IN_DIM, OUT_DIM = 2048, 8192, 8192
NCORES = 8
OUT_SH = OUT_DIM // NCORES      # 1024 outputs per core
NSLOT = OUT_SH // 128           # 8 partition-slots per core
CHUNKS = (2, 2, 2, 2)           # slots per dma_gather call (sum = NSLOT)
GATHER_ORDER = (0, 3, 1, 2)     # chunk gather issue order
COMPUTE_ORDER = (0, 1, 6, 7, 2, 3, 4, 5)  # slot compute order
NDVE_U = 2                      # last computed slots: whole chain on DVE

# value = c0 + c1*a + c2*b + c3*ab  for each of the 16 gates
GATE_C = np.array(
    [
        # c0  c1  c2  c3
        [0, 0, 0, 0],    # 0  False
        [0, 0, 0, 1],    # 1  a AND b
        [0, 1, 0, -1],   # 2  a AND NOT b
        [0, 1, 0, 0],    # 3  a
        [0, 0, 1, -1],   # 4  NOT a AND b
        [0, 0, 1, 0],    # 5  b
        [0, 1, 1, -2],   # 6  a XOR b
        [0, 1, 1, -1],   # 7  a OR b
        [1, -1, -1, 1],  # 8  NOT (a OR b)
        [1, -1, -1, 2],  # 9  NOT (a XOR b)
        [1, 0, -1, 0],   # 10 NOT b
        [1, 0, -1, 1],   # 11 a OR NOT b
        [1, -1, 0, 0],   # 12 NOT a
        [1, -1, 0, 1],   # 13 NOT a OR b
        [1, 0, 0, -1],   # 14 NOT (a AND b)
        [1, 0, 0, 0],    # 15 True
    ],
    dtype=np.float32,
)  # [16, 4]


# ---------------------------------------------------------------- device IR
def build_nc(B=B_TOT, IN=IN_DIM, OSH=OUT_SH):
    """Build the per-core Bass module (SPMD; all cores run the same IR)."""
    import sys

    if "/opt/trn_rl_repo" not in sys.path:
        sys.path.insert(0, "/opt/trn_rl_repo")

    import concourse.tile as tile
    from concourse import bacc, mybir
    from contextlib import ExitStack

    f32 = mybir.dt.float32
    f16 = mybir.dt.float16
    i16 = mybir.dt.int16
    u8 = mybir.dt.uint8

    nc = bacc.Bacc("TRN2", target_bir_lowering=False)
    xT = nc.declare_dram_parameter("xT", [IN, B], f16, isOutput=False)
    wgt = nc.declare_dram_parameter("wgt_shuf", [128, NSLOT * 16], f32, isOutput=False)
    cg = nc.declare_dram_parameter("cgate", [128, 64], f32, isOutput=False)
    idxa = nc.declare_dram_parameter("idxa16", [128, OSH // 16], i16, isOutput=False)
    idxb = nc.declare_dram_parameter("idxb16", [128, OSH // 16], i16, isOutput=False)
    outT = nc.declare_dram_parameter("outT", [OSH, B], f16, isOutput=True)

    Ident = mybir.ActivationFunctionType.Identity
    Exp = mybir.ActivationFunctionType.Exp
    MULT = mybir.AluOpType.mult
    ADD = mybir.AluOpType.add

    with tile.TileContext(nc) as tc, ExitStack() as ctx:
        cpool = ctx.enter_context(tc.tile_pool(name="consts", bufs=1))
        wpool = ctx.enter_context(tc.tile_pool(name="wtmp", bufs=2))
        gpool = ctx.enter_context(tc.tile_pool(name="gath", bufs=1, side="right"))
        upool = ctx.enter_context(tc.tile_pool(name="u", bufs=6))
        vpool = ctx.enter_context(tc.tile_pool(name="v", bufs=6))
        tpool = ctx.enter_context(tc.tile_pool(name="t", bufs=3))
        opool = ctx.enter_context(tc.tile_pool(name="o", bufs=4))

        # idx loads first: the gather chain is the critical path
        idxa_sb = cpool.tile([128, OSH // 16], i16, name="idxa_sb")
        nc.sync.dma_start(idxa_sb[:], idxa[:])
        idxb_sb = cpool.tile([128, OSH // 16], i16, name="idxb_sb")
        nc.sync.dma_start(idxb_sb[:], idxb[:])
        cgt = cpool.tile([128, 64], f32, name="cgt")
        nc.sync.dma_start(cgt[:], cg[:])
        wtile = wpool.tile([128, NSLOT * 16], f32, name="wtile")
        nc.sync.dma_start(wtile[:], wgt[:])

        # ---- gathers launch first (longest dependency chain) -------------
        # j = (s0 + c)*128 + p lands at ga[p, c, :]. The LAST-computed
        # chunks are gathered early (GATHER_ORDER) so the compute tail is
        # never data-starved and the kernel ends on the store stream.
        gt = {}
        for gi, ck in enumerate(GATHER_ORDER):
            CH = CHUNKS[ck]
            s0 = sum(CHUNKS[:ck])
            NJ = CH * 128
            ga = gpool.tile([128, CH, B], f16, name=f"ga{ck}", tag=f"ga{ck}")
            nc.gpsimd.dma_gather(
                ga[:], xT[:], idxa_sb[:, s0 * 8:(s0 + CH) * 8],
                NJ, NJ, B,
            )
            gb = gpool.tile([128, CH, B], f16, name=f"gb{ck}", tag=f"gb{ck}")
            nc.gpsimd.dma_gather(
                gb[:], xT[:], idxb_sb[:, s0 * 8:(s0 + CH) * 8],
                NJ, NJ, B,
            )
            gt[ck] = (ga, gb)

            # ---- W = softmax(weights) @ C, layout wk[k][q, r], j = r*128+q
            # (issued after the first chunk's gathers so SWDGE gen leads)
            if gi == 0:
                # W-phase reduces on GpSimd: it is idle until the first
                # dma_gather unblocks (fixed ~16.6us into the kernel)
                wexp = wpool.tile([128, NSLOT * 16], f32, name="wexp")
                nc.scalar.activation(wexp[:], wtile[:], Exp)
                wsum = wpool.tile([128, NSLOT], f32, name="wsum")
                nc.vector.tensor_reduce(
                    out=wsum[:],
                    in_=wexp[:].rearrange("p (r k) -> p r k", k=16),
                    op=ADD,
                    axis=mybir.AxisListType.X,
                )
                wrcp = wpool.tile([128, NSLOT], f32, name="wrcp")
                nc.vector.reciprocal(wrcp[:], wsum[:])
                wk = [cpool.tile([128, NSLOT], f32, name=f"wk{k}") for k in range(4)]
                for k in range(4):
                    wtmp = wpool.tile([128, NSLOT * 16], f32, name="wtmp", tag="wtmp")
                    ck_bcast = (
                        cgt[:, k * 16:(k + 1) * 16]
                        .rearrange("p (r k) -> p r k", r=1)
                        .to_broadcast([128, NSLOT, 16])
                    )
                    nc.vector.tensor_tensor(
                        out=wtmp[:].rearrange("p (r k) -> p r k", k=16),
                        in0=wexp[:].rearrange("p (r k) -> p r k", k=16),
                        in1=ck_bcast,
                        op=MULT,
                    )
                    wred = wpool.tile([128, NSLOT], f32, name="wred", tag="wred")
                    nc.vector.tensor_reduce(
                        out=wred[:],
                        in_=wtmp[:].rearrange("p (r k) -> p r k", k=16),
                        op=ADD,
                        axis=mybir.AxisListType.X,
                    )
                    nc.vector.tensor_tensor(out=wk[k][:], in0=wred[:], in1=wrcp[:],
                                            op=MULT)

        # ---- gates: out = (W1*a + W0) + (W3*a + W2)*b --------------------
        for ci, r in enumerate(COMPUTE_ORDER):
            ck, c = r // 2, r % 2
            ga, gb = gt[ck]
            if True:
                u = upool.tile([128, B], f16, tag="u")
                if ci >= len(COMPUTE_ORDER) - NDVE_U:
                    # tail slots: keep the whole chain on DVE so the final
                    # slots don't wait on ACT's slow serial u stream
                    nc.vector.tensor_scalar(
                        u[:], ga[:, c, :],
                        wk[3][:, r:r + 1], wk[2][:, r:r + 1],
                        op0=MULT, op1=ADD,
                    )
                else:
                    nc.scalar.activation(
                        u[:], ga[:, c, :], Ident,
                        scale=wk[3][:, r:r + 1], bias=wk[2][:, r:r + 1],
                    )
                v = vpool.tile([128, B], f16, tag="v")
                nc.vector.tensor_scalar(
                    v[:], ga[:, c, :],
                    wk[1][:, r:r + 1], wk[0][:, r:r + 1],
                    op0=MULT, op1=ADD,
                )
                t = tpool.tile([128, B], f16, tag="t")
                nc.vector.tensor_tensor(t[:], u[:], gb[:, c, :], op=MULT)
                # slot pairs (consecutive in COMPUTE_ORDER, same chunk) share
                # one store tile -> 4 stores of 1 MiB; issued from the Scalar
                # engine's HWDGE so they queue behind the u-pass stream and
                # yield the DMA bus to the remaining gathers
                if ci % 2 == 0:
                    po = opool.tile([128, 2, B], f16, tag="o")
                nc.vector.tensor_tensor(po[:, ci % 2, :], t[:], v[:], op=ADD)
                if ci % 2 == 1:
                    r0 = COMPUTE_ORDER[ci - 1]
                    assert r == r0 + 1
                    nc.scalar.dma_start(
                        outT[r0 * 128:(r0 + 2) * 128, :]
                        .rearrange("(i p) b -> p i b", p=128),
                        po[:],
                    )
    nc.compile()
    return nc


# ---------------------------------------------------------------- host side
def _wrap_idx(idx, n):
    """Pack an index vector into dma_gather's wrapped int16 layout.

    idx16[p, s] = idx[s*16 + p%16], replicated over the 8 groups of 16
    partitions; a gather of NJ indices starting at slot s0 then reads
    columns [s0*8, s0*8 + NJ/16).
    """
    a = np.asarray(idx).astype(np.int16).reshape(n // 16, 16)  # [s, p]
    a = np.ascontiguousarray(a.T)                              # [16, s]
    return np.ascontiguousarray(np.tile(a, (8, 1)))            # [128, s]


def _prep_inputs(x, weights, idx_a, idx_b):
    x = np.asarray(x, dtype=np.float32)
    weights = np.asarray(weights, dtype=np.float32)
    idx_a = np.asarray(idx_a)
    idx_b = np.asarray(idx_b)
    xT16 = np.ascontiguousarray(x.astype(np.float16).T)  # [IN, B] fp16
    cgate = np.ascontiguousarray(np.tile(GATE_C.T.reshape(1, 64), (128, 1)))
    in_maps = []
    for c in range(NCORES):
        j0 = c * OUT_SH
        wsh = weights[j0:j0 + OUT_SH]  # [1024, 16]
        # wgt_shuf[q, r*16+k] = weights[j0 + r*128 + q, k]
        wgt_shuf = np.ascontiguousarray(
            wsh.reshape(NSLOT, 128, 16).transpose(1, 0, 2).reshape(128, -1)
        )
        in_maps.append(
            {
                "xT": xT16,
                "wgt_shuf": wgt_shuf,
                "cgate": cgate,
                "idxa16": _wrap_idx(idx_a[j0:j0 + OUT_SH], OUT_SH),
                "idxb16": _wrap_idx(idx_b[j0:j0 + OUT_SH], OUT_SH),
            }
        )
    return in_maps


def _assemble(results):
    """[OUT_SH, B] fp16 per core -> full [B, OUT] f32."""
    stacked = np.stack([np.asarray(r["outT"]) for r in results])  # [8, 1024, 2048]
    return np.ascontiguousarray(
        stacked.astype(np.float32).transpose(2, 0, 1).reshape(B_TOT, OUT_DIM)
    )


_NC_CACHE = {}


def _get_nc():
    if "nc" not in _NC_CACHE:
        _NC_CACHE["nc"] = build_nc()
    return _NC_CACHE["nc"]


def kernel(x, weights, idx_a, idx_b):
    import sys

    if "/opt/trn_rl_repo" not in sys.path:
        sys.path.insert(0, "/opt/trn_rl_repo")
    from concourse.bass_utils import run_bass_kernel_spmd

    nc = _get_nc()
    in_maps = _prep_inputs(x, weights, idx_a, idx_b)
    res = run_bass_kernel_spmd(nc, in_maps, list(range(NCORES)))
    return _assemble(res.results)


if __name__ == "__main__":
    nc = build_nc()
    print("built OK")


# revision 30
# speedup vs baseline: 1.3011x; 1.0525x over previous
"""Trainium2 Bass kernel for nn_LogicLayer (differentiable logic-gate layer).

Reference computation:
    a = x[:, idx_a]; b = x[:, idx_b]                  # [B, OUT] gathers
    w = softmax(weights, -1)                          # [OUT, 16]
    out = sum_k w[:, k] * gate_k(a, b)

Every gate value is of the form c0 + c1*a + c2*b + c3*a*b, so
    out[i, j] = W0[j] + W1[j]*a + W2[j]*b + W3[j]*a*b
with W = softmax(weights) @ C, C the [16, 4] gate-coefficient table.

Kernel strategy (out_dim-parallel across 8 cores, 1024 outputs/core,
full 2048-row batch per core):
  - host passes xT = x.T as fp16 [8192, 2048]; each gathered row is then
    4 KiB, so a core needs only 2048 gather descriptors total (SWDGE
    descriptor generation at ~8.5 ns/desc was the old bottleneck)
  - softmax+C projection on device -> W0..W3 [128, 8] tiles in SBUF
  - dma_gather rows of xT for idx_a / idx_b; out_dim lands on partitions
    (j = slot*128 + p), batch on the free axis
  - u = W3*a + W2 (ACT), v = W1*a + W0 (DVE ts), t = u*b (DVE tt),
    o = t + v (DVE tt), everything fp16
  - o stores straight to DRAM as outT [1024, 2048] fp16; host transposes
    back to [2048, OUT] f32 and concatenates the 8 core slices
"""

import numpy as np

# ---------------------------------------------------------------- constants
B_TOT, IN_DIM, OUT_DIM = 2048, 8192, 8192
NCORES = 8
OUT_SH = OUT_DIM // NCORES      # 1024 outputs per core
NSLOT = OUT_SH // 128           # 8 partition-slots per core
CHUNKS = (2, 2, 2, 2)           # slots per dma_gather call (sum = NSLOT)
GATHER_ORDER = (0, 3, 1, 2)     # chunk gather issue order
COMPUTE_ORDER = (0, 1, 6, 7, 2, 3, 4, 5)  # slot compute order
NDVE_U = 2                      # last computed slots: whole chain on DVE

# value = c0 + c1*a + c2*b + c3*ab  for each of the 16 gates
GATE_C = np.array(
    [
        # c0  c1  c2  c3
        [0, 0, 0, 0],    # 0  False
        [0, 0, 0, 1],    # 1  a AND b
        [0, 1, 0, -1],   # 2  a AND NOT b
        [0, 1, 0, 0],    # 3  a
        [0, 0, 1, -1],   # 4  NOT a AND b
        [0, 0, 1, 0],    # 5  b
        [0, 1, 1, -2],   # 6  a XOR b
        [0, 1, 1, -1],   # 7  a OR b
        [1, -1, -1, 1],  # 8  NOT (a OR b)
        [1, -1, -1, 2],  # 9  NOT (a XOR b)
        [1, 0, -1, 0],   # 10 NOT b
        [1, 0, -1, 1],   # 11 a OR NOT b
        [1, -1, 0, 0],   # 12 NOT a
        [1, -1, 0, 1],   # 13 NOT a OR b
        [1, 0, 0, -1],   # 14 NOT (a AND b)
        [1, 0, 0, 0],    # 15 True
    ],
    dtype=np.float32,
)  # [16, 4]


# ---------------------------------------------------------------- device IR
def build_nc(B=B_TOT, IN=IN_DIM, OSH=OUT_SH):
    """Build the per-core Bass module (SPMD; all cores run the same IR)."""
    import sys

    if "/opt/trn_rl_repo" not in sys.path:
        sys.path.insert(0, "/opt/trn_rl_repo")

    import concourse.tile as tile
    from concourse import bacc, mybir
    from contextlib import ExitStack

    f32 = mybir.dt.float32
    f16 = mybir.dt.float16
    i16 = mybir.dt.int16
    u8 = mybir.dt.uint8

    nc = bacc.Bacc("TRN2", target_bir_lowering=False, num_swdge_queues=2)
    xT = nc.declare_dram_parameter("xT", [IN, B], f16, isOutput=False)
    wgt = nc.declare_dram_parameter("wgt_shuf", [128, NSLOT * 16], f32, isOutput=False)
    cg = nc.declare_dram_parameter("cgate", [128, 64], f32, isOutput=False)
    idxa = nc.declare_dram_parameter("idxa16", [128, OSH // 16], i16, isOutput=False)
    idxb = nc.declare_dram_parameter("idxb16", [128, OSH // 16], i16, isOutput=False)
    outT = nc.declare_dram_parameter("outT", [OSH, B], f16, isOutput=True)

    Ident = mybir.ActivationFunctionType.Identity
    Exp = mybir.ActivationFunctionType.Exp
    MULT = mybir.AluOpType.mult
    ADD = mybir.AluOpType.add

    with tile.TileContext(nc) as tc, ExitStack() as ctx:
        cpool = ctx.enter_context(tc.tile_pool(name="consts", bufs=1))
        wpool = ctx.enter_context(tc.tile_pool(name="wtmp", bufs=2))
        gpool = ctx.enter_context(tc.tile_pool(name="gath", bufs=1, side="right"))
        upool = ctx.enter_context(tc.tile_pool(name="u", bufs=6))
        vpool = ctx.enter_context(tc.tile_pool(name="v", bufs=6))
        tpool = ctx.enter_context(tc.tile_pool(name="t", bufs=3))
        opool = ctx.enter_context(tc.tile_pool(name="o", bufs=4))

        # idx loads first: the gather chain is the critical path
        idxa_sb = cpool.tile([128, OSH // 16], i16, name="idxa_sb")
        nc.sync.dma_start(idxa_sb[:], idxa[:])
        idxb_sb = cpool.tile([128, OSH // 16], i16, name="idxb_sb")
        nc.sync.dma_start(idxb_sb[:], idxb[:])
        cgt = cpool.tile([128, 64], f32, name="cgt")
        nc.sync.dma_start(cgt[:], cg[:])
        wtile = wpool.tile([128, NSLOT * 16], f32, name="wtile")
        nc.sync.dma_start(wtile[:], wgt[:])

        # ---- gathers launch first (longest dependency chain) -------------
        # j = (s0 + c)*128 + p lands at ga[p, c, :]. The LAST-computed
        # chunks are gathered early (GATHER_ORDER) so the compute tail is
        # never data-starved and the kernel ends on the store stream.
        gt = {}
        for gi, ck in enumerate(GATHER_ORDER):
            CH = CHUNKS[ck]
            s0 = sum(CHUNKS[:ck])
            NJ = CH * 128
            ga = gpool.tile([128, CH, B], f16, name=f"ga{ck}", tag=f"ga{ck}")
            nc.gpsimd.dma_gather(
                ga[:], xT[:], idxa_sb[:, s0 * 8:(s0 + CH) * 8],
                NJ, NJ, B,
            )
            gb = gpool.tile([128, CH, B], f16, name=f"gb{ck}", tag=f"gb{ck}")
            nc.gpsimd.dma_gather(
                gb[:], xT[:], idxb_sb[:, s0 * 8:(s0 + CH) * 8],
                NJ, NJ, B, queue_num=1,
            )
            gt[ck] = (ga, gb)

            # ---- W = softmax(weights) @ C, layout wk[k][q, r], j = r*128+q
            # (issued after the first chunk's gathers so SWDGE gen leads)
            if gi == 0:
                # W-phase reduces on GpSimd: it is idle until the first
                # dma_gather unblocks (fixed ~16.6us into the kernel)
                wexp = wpool.tile([128, NSLOT * 16], f32, name="wexp")
                nc.scalar.activation(wexp[:], wtile[:], Exp)
                wsum = wpool.tile([128, NSLOT], f32, name="wsum")
                nc.vector.tensor_reduce(
                    out=wsum[:],
                    in_=wexp[:].rearrange("p (r k) -> p r k", k=16),
                    op=ADD,
                    axis=mybir.AxisListType.X,
                )
                wrcp = wpool.tile([128, NSLOT], f32, name="wrcp")
                nc.vector.reciprocal(wrcp[:], wsum[:])
                wk = [cpool.tile([128, NSLOT], f32, name=f"wk{k}") for k in range(4)]
                for k in range(4):
                    wtmp = wpool.tile([128, NSLOT * 16], f32, name="wtmp", tag="wtmp")
                    ck_bcast = (
                        cgt[:, k * 16:(k + 1) * 16]
                        .rearrange("p (r k) -> p r k", r=1)
                        .to_broadcast([128, NSLOT, 16])
                    )
                    nc.vector.tensor_tensor(
                        out=wtmp[:].rearrange("p (r k) -> p r k", k=16),
                        in0=wexp[:].rearrange("p (r k) -> p r k", k=16),
                        in1=ck_bcast,
                        op=MULT,
                    )
                    wred = wpool.tile([128, NSLOT], f32, name="wred", tag="wred")
                    nc.vector.tensor_reduce(
                        out=wred[:],
                        in_=wtmp[:].rearrange("p (r k) -> p r k", k=16),
                        op=ADD,
                        axis=mybir.AxisListType.X,
                    )
                    nc.vector.tensor_tensor(out=wk[k][:], in0=wred[:], in1=wrcp[:],
                                            op=MULT)

        # ---- gates: out = (W1*a + W0) + (W3*a + W2)*b --------------------
        for ci, r in enumerate(COMPUTE_ORDER):
            ck, c = r // 2, r % 2
            ga, gb = gt[ck]
            if True:
                u = upool.tile([128, B], f16, tag="u")
                if ci >= len(COMPUTE_ORDER) - NDVE_U:
                    # tail slots: keep the whole chain on DVE so the final
                    # slots don't wait on ACT's slow serial u stream
                    nc.vector.tensor_scalar(
                        u[:], ga[:, c, :],
                        wk[3][:, r:r + 1], wk[2][:, r:r + 1],
                        op0=MULT, op1=ADD,
                    )
                else:
                    nc.scalar.activation(
                        u[:], ga[:, c, :], Ident,
                        scale=wk[3][:, r:r + 1], bias=wk[2][:, r:r + 1],
                    )
                v = vpool.tile([128, B], f16, tag="v")
                nc.vector.tensor_scalar(
                    v[:], ga[:, c, :],
                    wk[1][:, r:r + 1], wk[0][:, r:r + 1],
                    op0=MULT, op1=ADD,
                )
                t = tpool.tile([128, B], f16, tag="t")
                nc.vector.tensor_tensor(t[:], u[:], gb[:, c, :], op=MULT)
                # slot pairs (consecutive in COMPUTE_ORDER, same chunk) share
                # one store tile -> 4 stores of 1 MiB; issued from the Scalar
                # engine's HWDGE so they queue behind the u-pass stream and
                # yield the DMA bus to the remaining gathers
                if ci % 2 == 0:
                    po = opool.tile([128, 2, B], f16, tag="o")
                nc.vector.tensor_tensor(po[:, ci % 2, :], t[:], v[:], op=ADD)
                if ci % 2 == 1:
                    r0 = COMPUTE_ORDER[ci - 1]
                    assert r == r0 + 1
                    nc.scalar.dma_start(
                        outT[r0 * 128:(r0 + 2) * 128, :]
                        .rearrange("(i p) b -> p i b", p=128),
                        po[:],
                    )
    nc.compile()
    return nc


# ---------------------------------------------------------------- host side
def _wrap_idx(idx, n):
    """Pack an index vector into dma_gather's wrapped int16 layout.

    idx16[p, s] = idx[s*16 + p%16], replicated over the 8 groups of 16
    partitions; a gather of NJ indices starting at slot s0 then reads
    columns [s0*8, s0*8 + NJ/16).
    """
    a = np.asarray(idx).astype(np.int16).reshape(n // 16, 16)  # [s, p]
    a = np.ascontiguousarray(a.T)                              # [16, s]
    return np.ascontiguousarray(np.tile(a, (8, 1)))            # [128, s]


def _prep_inputs(x, weights, idx_a, idx_b):
    x = np.asarray(x, dtype=np.float32)
    weights = np.asarray(weights, dtype=np.float32)
    idx_a = np.asarray(idx_a)
    idx_b = np.asarray(idx_b)
    xT16 = np.ascontiguousarray(x.astype(np.float16).T)  # [IN, B] fp16
    cgate = np.ascontiguousarray(np.tile(GATE_C.T.reshape(1, 64), (128, 1)))
    in_maps = []
    for c in range(NCORES):
        j0 = c * OUT_SH
        wsh = weights[j0:j0 + OUT_SH]  # [1024, 16]
        # wgt_shuf[q, r*16+k] = weights[j0 + r*128 + q, k]
        wgt_shuf = np.ascontiguousarray(
            wsh.reshape(NSLOT, 128, 16).transpose(1, 0, 2).reshape(128, -1)
        )
        in_maps.append(
            {
                "xT": xT16,
                "wgt_shuf": wgt_shuf,
                "cgate": cgate,
                "idxa16": _wrap_idx(idx_a[j0:j0 + OUT_SH], OUT_SH),
                "idxb16": _wrap_idx(idx_b[j0:j0 + OUT_SH], OUT_SH),
            }
        )
    return in_maps


def _assemble(results):
    """[OUT_SH, B] fp16 per core -> full [B, OUT] f32."""
    stacked = np.stack([np.asarray(r["outT"]) for r in results])  # [8, 1024, 2048]
    return np.ascontiguousarray(
        stacked.astype(np.float32).transpose(2, 0, 1).reshape(B_TOT, OUT_DIM)
    )


_NC_CACHE = {}


def _get_nc():
    if "nc" not in _NC_CACHE:
        _NC_CACHE["nc"] = build_nc()
    return _NC_CACHE["nc"]


def kernel(x, weights, idx_a, idx_b):
    import sys

    if "/opt/trn_rl_repo" not in sys.path:
        sys.path.insert(0, "/opt/trn_rl_repo")
    from concourse.bass_utils import run_bass_kernel_spmd

    nc = _get_nc()
    in_maps = _prep_inputs(x, weights, idx_a, idx_b)
    res = run_bass_kernel_spmd(nc, in_maps, list(range(NCORES)))
    return _assemble(res.results)


if __name__ == "__main__":
    nc = build_nc()
    print("built OK")
